# revision 27
# baseline (speedup 1.0000x reference)
"""Trainium2 Bass kernel for nn_AttentionHead_Hybrid2 (B=4, N=4096, DK=64).

reference:
    V = x @ Wv.T + bv              (B,N,DK)
    Q = x @ wq ; K = x @ wk        (B,N)
    A = exp(-(Q_i - K_j)^2)        (B,N,N)
    P = softmax(A / 8, axis=-1)
    out = LN(P @ V + x)

Sharding: 8 cores = (batch b = c//2) x (query half c%2). Each core gets the
full key set for its batch (rolled so its 2048 queries are rows 0:2048) and
produces its 2048x64 output slice.

Algorithm (Fourier separation): the score E(q,k) = exp(exp(-(q-k)^2)/8)
depends only on t = q - k, so it has a rapidly-converging cosine expansion
E(t) = sum_k a_k cos(w_k t) (periodized, L=13, 24 cos/sin features gives
~3e-5 abs accuracy). cos(w(Q-K)) = cosQcosK + sinQsinK makes attention
separable with rank 24:
    num (2048, 65) = (PhiQ*a) (2048,24) @ [ Wv-transformed PhiK-moments ]
where PhiK/PhiQ are sin/cos feature maps of the key/query scalar
projections; col 64 = softmax denominator. No (N,N) scores. The Fourier
coefficients a ride along in the PhiQ transpose-copies (tensor_scalar).

LayerNorm handling: LN is scale-invariant, so z = num + den*x needs no
division, and the LN mean is *exactly zero* by construction: the host
pre-centers wvb's value columns (so num rows sum to 0) and ships
pre-centered queries xc = x - rowmean(x) (so den*x sums to ~0). Then
out = z * rsqrt(sum(z^2)/64): no mu pipeline at all.

Phases are computed in turns r = u/2pi directly from x: per 128-token tile,
u_tile = xth_tile.T @ W2 with W2 = w (x) k/L + phase-row (bf16 hi/lo split).
The ACT sin table is valid on [-pi,pi]; a magic-number pass forms
w' = round(r) - r in [-0.5,0.5] and the ACT evaluates sin(2pi*w'). w' is the
NEGATED reduced phase - the sign cancels in the PhiQ.PhiK quadratic form,
and this direction lets the ACT produce round() (Identity + MAGIC bias,
rounding in the fp32 output write) with a single DVE combine after.

DMA: six large input transfers (>=1KB per partition line) split across
three descriptor-generation queues (sync HWDGE, scalar HWDGE, gpsimd
SWDGE) since each dma_start costs ~0.7us of serialized descriptor-gen on
its issuing sequencer. First matmul starts ~2us in.
"""

import math
import sys

for _p in ("/opt/trn_rl_repo", "/root/.axon_site/_ro/trn_rl_repo"):
    if _p not in sys.path:
        sys.path.insert(0, _p)

import numpy as np

import concourse.bass as bass
import concourse.mybir as mybir
import concourse.tile as tile
import bass_rust
from concourse.bass_utils import run_bass_kernel_spmd

F32 = mybir.dt.float32
BF16 = mybir.dt.bfloat16
AF = mybir.ActivationFunctionType
OP = mybir.AluOpType

B, N, DK = 4, 4096, 64
NQ = 2048          # queries per core
NCORES = 8
JT = N // 128      # 32 key tiles
IT = NQ // 128     # 16 query tiles
LPER = 13.0        # Fourier period in t = q - k
NF = 24            # features: cos k=0..12, sin k=1..11
MAGIC = float(np.float32(1.5 * 2 ** 23))   # fp32 round-to-nearest trick
GRP = 16           # tiles per phase group

# consts column layout (bf16, 128 x 512)
_WF0 = 0            # (65, 96)  w2kh|w2kl|w2qh|w2ql
_IDB0 = 96          # (128, 128) identity
_WVH0 = 224         # (65, 65)  wvb hi
_WVL0 = 289         # (65, 65)  wvb lo
_ACH = 354          # (24, 1)   a hi
_ACL = 355          # (24, 1)   a lo
_MAG = 356          # (128, 1)  MAGIC (bf16-exact)
CONSTS_W = 512


def split_multiwaits(nc):
    """Walrus in this env accepts one sem-wait per instruction; Tile emits
    several. Split extras onto preceding same-engine NoOps."""
    ctr = 0
    for f in nc.m.functions:
        for bb in f.blocks:
            out, changed = [], False
            for ins in bb.instructions:
                si = ins.sync_info
                if si is not None and si.on_wait and len(si.on_wait) > 1:
                    waits = list(si.on_wait)
                    for w in waits[:-1]:
                        ctr += 1
                        out.append(mybir.InstNoOp(
                            name=f"I-wsplit-{ctr}", engine=ins.engine,
                            debug=ins.debug, ins=[], outs=[],
                            sync_info=bass_rust.SyncInfo(on_wait=[w], on_update=[])))
                    ins.sync_info = bass_rust.SyncInfo(
                        on_wait=[waits[-1]], on_update=list(si.on_update or []))
                    changed = True
                out.append(ins)
            if changed:
                bb.instructions = out
    return ctr


def build_nc(split=True):
    nc = bass.Bass("TRN2", target_bir_lowering=False, debug=False)

    consts_d = nc.dram_tensor("consts", [128, CONSTS_W], BF16,
                              kind="ExternalInput").ap()
    xth_d = nc.dram_tensor("xth", [DK + 1, N], BF16, kind="ExternalInput").ap()
    xa_d = nc.dram_tensor("xa", [128, JT * 65], BF16, kind="ExternalInput").ap()
    xc_d = nc.dram_tensor("xc", [128, IT * DK], BF16, kind="ExternalInput").ap()
    out_d = nc.dram_tensor("out", [128, IT * DK], F32, kind="ExternalOutput").ap()

    with tile.TileContext(nc) as tc:
        cpool = tc.alloc_tile_pool(name="consts", bufs=1)
        big = tc.alloc_tile_pool(name="big", bufs=1)

        # ---- input DMAs across three descriptor-gen queues ----
        consts = cpool.tile([128, CONSTS_W], BF16)
        xth = big.tile([DK + 1, N], BF16)
        # sync ring: consts (first matmul needs w2), then half A (queries)
        nc.sync.dma_start(consts[:], consts_d[:])
        nc.sync.dma_start(xth[:, 0:1024], xth_d[:, 0:1024])
        nc.sync.dma_start(xth[:, 1024:2048], xth_d[:, 1024:2048])
        # scalar (ACT) HWDGE ring in parallel: key half B
        nc.scalar.dma_start(xth[:, 2048:3072], xth_d[:, 2048:3072])
        nc.scalar.dma_start(xth[:, 3072:4096], xth_d[:, 3072:4096])
        # gpsimd SWDGE ring; hold xa/xc back until the xth half-B data has
        # landed so xth/consts get the full HBM bandwidth first (WAR dep on
        # one element delays the descriptor-gen)
        xa_all = big.tile([128, JT * 65], BF16)
        xa_v = xa_all.rearrange("p (t c) -> p t c", c=65)
        xc_all = big.tile([128, IT * DK], BF16)
        xc_v = xc_all.rearrange("p (t d) -> p t d", d=DK)
        nc.vector.tensor_copy(xa_all[0:1, 0:1], xth[0:1, 1024:1025])
        nc.gpsimd.dma_start(xa_all[:], xa_d[:])
        nc.gpsimd.dma_start(xc_all[:], xc_d[:])

        wf = consts[0:65, _WF0:_WF0 + 96]
        w2kh = wf[:, 0 * NF:1 * NF]
        w2kl = wf[:, 1 * NF:2 * NF]
        w2qh = wf[:, 2 * NF:3 * NF]
        w2ql = wf[:, 3 * NF:4 * NF]
        idb = consts[:, _IDB0:_IDB0 + 128]
        magic = consts[:, _MAG:_MAG + 1]

        # ---- SBUF working set ----
        phk = big.tile([128, JT * NF], BF16)       # key features, tile-major
        phk_v = phk.rearrange("p (t f) -> p t f", f=NF)
        phqt = big.tile([128, IT * NF], BF16)      # query features, tile-major
        phqt_v = phqt.rearrange("p (t f) -> p t f", f=NF)
        phq = big.tile([NF, NQ], BF16)             # a * PhiQ.T (feat, token)
        w_sb = big.tile([128, 3 * GRP * NF], F32)  # w' = round(u) - u
        w_v = w_sb.rearrange("p (g c) -> p g c", c=GRP * NF)
        wvb = big.tile([DK + 1, 65], F32)          # reconstructed fp32 wvb
        acol = big.tile([NF, 1], F32)
        f_sb = big.tile([DK + 1, NF], F32)         # key feature moments
        fwh = big.tile([NF, 65], BF16)
        z_sb = big.tile([128, IT * DK], F32)
        z_v = z_sb.rearrange("p (t d) -> p t d", d=DK)
        sq = big.tile([128, IT * DK], F32)
        sq_v = sq.rearrange("p (t d) -> p t d", d=DK)
        t1 = big.tile([128, 2 * 4 * DK], F32)
        t1_v = t1.rearrange("p (t d) -> p t d", d=DK)
        s2 = big.tile([128, IT], F32)
        rstd = big.tile([128, IT], F32)
        o_sb = big.tile([128, IT * DK], F32)
        o_v = o_sb.rearrange("p (t d) -> p t d", d=DK)
        dummy = big.tile([1, 2], F32)

        nc.gpsimd.memset(dummy[:], 0.0)
        # prefetch the trig ACT table at t~0 (Sin(0) is in-range)
        nc.scalar.activation(dummy[:, 0:1], dummy[:, 0:1], AF.Sin, scale=1.0)
        # reconstruct fp32 consts from bf16 hi/lo (off the critical path)
        nc.vector.tensor_tensor(wvb[:], consts[0:65, _WVH0:_WVH0 + 65],
                                consts[0:65, _WVL0:_WVL0 + 65], OP.add)
        nc.vector.tensor_tensor(acol[:], consts[0:NF, _ACH:_ACH + 1],
                                consts[0:NF, _ACL:_ACL + 1], OP.add)

        with (tc.tile_pool(name="u_ps", bufs=3, space="PSUM") as ups,
              tc.tile_pool(name="f_ps", bufs=1, space="PSUM") as fps,
              tc.tile_pool(name="pt_ps", bufs=2, space="PSUM") as ptps,
              tc.tile_pool(name="fw_ps", bufs=1, space="PSUM") as fwps,
              tc.tile_pool(name="wm_ps", bufs=1, space="PSUM") as wmps,
              tc.tile_pool(name="rt_sb", bufs=3) as rtp):
            # PE warmup: tiny dependent matmuls keep the PE's HAM activity
            # window busy during the input-DMA wait so the real matmuls run
            # at 2.4 GHz instead of the cold 1.2 GHz
            warm = wmps.tile([1, 1], F32, tag="w")
            for _ in range(70):
                nc.tensor.matmul(warm[:], dummy[0:1, 0:1], dummy[0:1, 0:1],
                                 start=True, stop=True)

            # ---- phase matmuls ----
            # g1: keys 0..15, g2: queries (same xth tiles, half A, loads
            # first), g0: keys 16..31 (half B)
            u1 = ups.tile([128, GRP * NF], F32, tag="u")
            u1_t = u1.rearrange("p (t f) -> p t f", f=NF)
            u2 = ups.tile([128, GRP * NF], F32, tag="u")
            u2_t = u2.rearrange("p (t f) -> p t f", f=NF)
            for i in range(GRP):
                sl = xth[:, i * 128:(i + 1) * 128]
                nc.tensor.matmul(u2_t[:, i, :], sl, w2qh, start=True, stop=False)
                nc.tensor.matmul(u2_t[:, i, :], sl, w2ql, start=False, stop=True)
                nc.tensor.matmul(u1_t[:, i, :], sl, w2kh, start=True, stop=False)
                nc.tensor.matmul(u1_t[:, i, :], sl, w2kl, start=False, stop=True)
            u0 = ups.tile([128, GRP * NF], F32, tag="u")
            u0_t = u0.rearrange("p (t f) -> p t f", f=NF)
            for i in range(GRP):
                sl = xth[:, (GRP + i) * 128:(GRP + i + 1) * 128]
                nc.tensor.matmul(u0_t[:, i, :], sl, w2kh, start=True, stop=False)
                nc.tensor.matmul(u0_t[:, i, :], sl, w2kl, start=False, stop=True)

            f_ps = fps.tile([DK + 1, NF], F32, tag="f")

            # ---- rounds + sins, in dependency-readiness order ----
            # g2 (queries) first: feeds the longest chain (transpose->phq)
            rt2 = rtp.tile([128, GRP * NF], F32, tag="rt")
            nc.vector.tensor_scalar(rt2[:], u2[:], MAGIC, MAGIC, OP.add,
                                    OP.subtract)
            nc.vector.tensor_tensor(w_v[:, 2, :], rt2[:], u2[:], OP.subtract)
            HW_ = GRP * NF // 2
            nc.scalar.activation(phqt[:, 0:HW_], w_v[:, 2, 0:HW_], AF.Sin,
                                 scale=2 * math.pi)
            nc.scalar.activation(phqt[:, HW_:2 * HW_], w_v[:, 2, HW_:2 * HW_],
                                 AF.Sin, scale=2 * math.pi)
            # g1 (keys 0..15): round on ACT, combine on DVE
            rt1 = rtp.tile([128, GRP * NF], F32, tag="rt")
            nc.scalar.activation(rt1[:], u1[:], AF.Identity, bias=magic,
                                 scale=1.0)
            nc.vector.scalar_tensor_tensor(
                w_v[:, 1, :], rt1[:], MAGIC, u1[:], OP.subtract, OP.subtract)
            nc.scalar.activation(phk[:, 0:GRP * NF], w_v[:, 1, :],
                                 AF.Sin, scale=2 * math.pi)
            for jt in range(GRP):
                nc.tensor.matmul(f_ps[:], xa_v[:, jt, :], phk_v[:, jt, :],
                                 start=(jt == 0), stop=False)
            # g0 (keys 16..31)
            rt0 = rtp.tile([128, GRP * NF], F32, tag="rt")
            nc.scalar.activation(rt0[:], u0[:], AF.Identity, bias=magic,
                                 scale=1.0)
            nc.vector.scalar_tensor_tensor(
                w_v[:, 0, :], rt0[:], MAGIC, u0[:], OP.subtract, OP.subtract)
            nc.scalar.activation(phk[:, GRP * NF:2 * GRP * NF], w_v[:, 0, :],
                                 AF.Sin, scale=2 * math.pi)
            for jt in range(GRP, JT):
                nc.tensor.matmul(f_ps[:], xa_v[:, jt, :], phk_v[:, jt, :],
                                 start=False, stop=(jt == JT - 1))

            # ---- moments -> fw (wvb pre-centered on host) ----
            nc.scalar.activation(f_sb[:], f_ps[:], AF.Identity, scale=1.0)
            fw_ps = fwps.tile([NF, 65], F32, tag="fw")
            nc.tensor.matmul(fw_ps[:], f_sb[:], wvb[:], start=True, stop=True)
            nc.scalar.activation(fwh[:], fw_ps[:], AF.Identity, scale=1.0)

            # ---- transpose query features to (24, NQ), scaling by a ----
            # (the a-coefficients ride along in the PSUM->SBUF copies)
            for g in range(2):
                pt = ptps.tile([NF, 1024], BF16, tag="pt")
                for i in range(8):
                    nc.tensor.transpose(pt[:, i * 128:(i + 1) * 128],
                                        phqt_v[:, g * 8 + i, :], idb)
                nc.vector.tensor_scalar(phq[:, g * 1024:(g + 1) * 1024],
                                        pt[:], acol[:], None, OP.mult)
            # trigger the trig -> ln/exp ACT table switch after the last
            # pre-switch ACT op (input depends on it: cannot be hoisted)
            nc.scalar.activation(dummy[:, 1:2], phk[0:1, 384:385], AF.Exp,
                                 scale=1.0)

        # ---- numerator + fused LN tail (mean is zero by construction) ----
        with tc.tile_pool(name="num_ps", bufs=4, space="PSUM") as nps:
            nvs = []
            for h in range(4):
                nf = nps.tile([128, 4 * 65], F32, tag="nf")
                nf_v = nf.rearrange("p (t c) -> p t c", c=65)
                nvs.append(nf_v)
                for j in range(4):
                    it = h * 4 + j
                    lhs = phq[:, it * 128:(it + 1) * 128]
                    nc.tensor.matmul(nf_v[:, j, :], lhs, fwh[:],
                                     start=True, stop=True)
            # per-chunk z and squares (ACT order: all squares, then ln/exp)
            for h in range(4):
                ts_, te_ = h * 4, (h + 1) * 4
                nv = nvs[h]
                tv = t1_v[:, (h % 2) * 4:(h % 2) * 4 + 4, :]
                # z = num + den * xc
                nc.vector.tensor_tensor(
                    tv, xc_v[:, ts_:te_, :],
                    nv[:, :, 64:65].broadcast_to([128, 4, DK]), OP.mult)
                nc.vector.tensor_tensor(z_v[:, ts_:te_, :], tv,
                                        nv[:, :, 0:64], OP.add)
                nc.scalar.activation(sq[:, ts_ * DK:te_ * DK],
                                     z_sb[:, ts_ * DK:te_ * DK],
                                     AF.Square, scale=1.0)
                nc.vector.reduce_sum(s2[:, ts_:te_], sq_v[:, ts_:te_, :],
                                     axis=mybir.AxisListType.X)
            for s in range(2):
                # rstd = exp(-0.5 * ln(s2/64)) = rsqrt(var)
                nc.scalar.activation(rstd[:, s * 8:(s + 1) * 8],
                                     s2[:, s * 8:(s + 1) * 8],
                                     AF.Ln, scale=1.0 / DK)
                nc.scalar.activation(rstd[:, s * 8:(s + 1) * 8],
                                     rstd[:, s * 8:(s + 1) * 8],
                                     AF.Exp, scale=-0.5)
                for hh in (2 * s, 2 * s + 1):
                    t0_, t1_ = hh * 4, (hh + 1) * 4
                    nc.vector.tensor_tensor(
                        o_v[:, t0_:t1_, :], z_v[:, t0_:t1_, :],
                        rstd[:, t0_:t1_].unsqueeze(-1).broadcast_to(
                            [128, 4, DK]), OP.mult)
                    eng = nc.gpsimd if hh % 2 == 0 else nc.sync
                    eng.dma_start(out_d[:, t0_ * DK:t1_ * DK],
                                  o_sb[:, t0_ * DK:t1_ * DK])

        big.release()
        cpool.release()

    if split:
        split_multiwaits(nc)
    return nc


_NC_CACHE = None


def _get_nc():
    global _NC_CACHE
    if _NC_CACHE is None:
        _NC_CACHE = build_nc()
    return _NC_CACHE


def _fourier_coeffs():
    m = 16384
    t = LPER * np.arange(m) / m
    tw = np.minimum(t, LPER - t)
    g = np.exp(np.exp(-tw ** 2) / 8.0) - 1.0
    c = np.fft.rfft(g) / m
    a_cos = np.concatenate([[1.0 + np.real(c[0])], 2 * np.real(c[1:13])])
    a_sin = 2 * np.real(c[1:12])
    return np.concatenate([a_cos, a_sin]).astype(np.float32)


def make_in_maps(x, Wv, bv, wq, wk, gamma, beta):
    import ml_dtypes
    bf = ml_dtypes.bfloat16
    x = np.asarray(x, np.float32)
    kfeat = np.concatenate([np.arange(13), np.arange(1, 12)]).astype(np.float64)
    phip = np.concatenate([0.25 * np.ones(13), np.zeros(11)])

    wvb = np.zeros((65, 65), np.float64)
    wvb[:64, :64] = np.asarray(Wv, np.float64).T
    wvb[64, :64] = np.asarray(bv, np.float64)
    wvb[64, 64] = 1.0
    # pre-center value columns so num rows sum to zero (LN mean trick)
    wvb[:, 0:64] -= wvb[:, 0:64].mean(axis=1, keepdims=True)
    wvb = wvb.astype(np.float32)

    def hilo(v):
        hi = v.astype(bf)
        lo = (v - hi.astype(np.float32)).astype(bf)
        return hi, lo

    def w2pair(w):
        full = np.concatenate(
            [np.outer(np.asarray(w, np.float64), kfeat / LPER),
             phip[None, :]], 0).astype(np.float32)
        return hilo(full)

    wkh, wkl = w2pair(wk)
    wqh, wql = w2pair(wq)
    a = _fourier_coeffs()
    ah, al = hilo(a)
    wvh, wvl = hilo(wvb)

    consts = np.zeros((128, CONSTS_W), bf)
    consts[0:65, _WF0:_WF0 + 96] = np.concatenate([wkh, wkl, wqh, wql], 1)
    consts[0:128, _IDB0:_IDB0 + 128] = np.eye(128, dtype=bf)
    consts[0:65, _WVH0:_WVH0 + 65] = wvh
    consts[0:65, _WVL0:_WVL0 + 65] = wvl
    consts[0:NF, _ACH] = ah
    consts[0:NF, _ACL] = al
    consts[:, _MAG] = bf(MAGIC)

    ones = np.ones((N, 1), np.float32)
    in_maps = []
    for c in range(NCORES):
        b, qoff = c // 2, (c % 2) * NQ
        xr = np.concatenate([x[b, qoff:], x[b, :qoff]], axis=0) if qoff else x[b]
        xth = np.concatenate([xr.T, ones.T], 0).astype(bf)
        xa = np.concatenate([xr, ones], 1).astype(bf)            # (N, 65)
        xcf = xr[0:NQ] - xr[0:NQ].mean(axis=1, keepdims=True)
        xc = xcf.astype(bf)                                      # (NQ, 64)
        # pre-tile to [p, tile, col] so device DMAs are contiguous
        xa_t = np.ascontiguousarray(
            xa.reshape(JT, 128, 65).transpose(1, 0, 2).reshape(128, JT * 65))
        xc_t = np.ascontiguousarray(
            xc.reshape(IT, 128, DK).transpose(1, 0, 2).reshape(128, IT * DK))
        in_maps.append({"xth": np.ascontiguousarray(xth),
                        "xa": xa_t, "xc": xc_t, "consts": consts})
    return in_maps


def kernel(x, Wv, bv, wq, wk, gamma, beta, _trace=False, _trace_cores=None):
    nc = _get_nc()
    in_maps = make_in_maps(x, Wv, bv, wq, wk, gamma, beta)
    res = run_bass_kernel_spmd(nc, in_maps, core_ids=list(range(NCORES)),
                               trace=_trace, trace_cores=_trace_cores)
    out = np.empty((B, N, DK), np.float32)
    for c in range(NCORES):
        b, qoff = c // 2, (c % 2) * NQ
        oc = res.results[c]["out"].reshape(128, IT, DK).transpose(1, 0, 2)
        out[b, qoff:qoff + NQ] = oc.reshape(NQ, DK)
    # gamma/beta are ones/zeros in this problem's setup; apply on host if not.
    g = np.asarray(gamma, np.float32)
    bt = np.asarray(beta, np.float32)
    if not (np.all(g == 1.0) and np.all(bt == 0.0)):
        out = out * g + bt
    kernel._last_results = res
    return out


# revision 29
# speedup vs baseline: 1.1131x; 1.1131x over previous
"""Trainium2 Bass kernel for nn_AttentionHead_Hybrid2 (B=4, N=4096, DK=64).

reference:
    V = x @ Wv.T + bv              (B,N,DK)
    Q = x @ wq ; K = x @ wk        (B,N)
    A = exp(-(Q_i - K_j)^2)        (B,N,N)
    P = softmax(A / 8, axis=-1)
    out = LN(P @ V + x)

Sharding: 8 cores = (batch b = c//2) x (query half c%2). Each core gets the
full key set for its batch (rolled so its 2048 queries are rows 0:2048) and
produces its 2048x64 output slice.

Algorithm (Fourier separation): the score E(q,k) = exp(exp(-(q-k)^2)/8)
depends only on t = q - k, so it has a rapidly-converging cosine expansion
E(t) = sum_k a_k cos(w_k t) (periodized, L=13, 24 cos/sin features gives
~3e-5 abs accuracy). cos(w(Q-K)) = cosQcosK + sinQsinK makes attention
separable with rank 24:
    num (2048, 65) = (PhiQ*a) (2048,24) @ [ Wv-transformed PhiK-moments ]
where PhiK/PhiQ are sin/cos feature maps of the key/query scalar
projections; col 64 = softmax denominator. No (N,N) scores. The Fourier
coefficients a ride along in the PhiQ transpose-copies (tensor_scalar).

LayerNorm handling: LN is scale-invariant, so z = num + den*x needs no
division, and the LN mean is *exactly zero* by construction: the host
pre-centers wvb's value columns (so num rows sum to 0) and ships
pre-centered queries xc = x - rowmean(x) (so den*x sums to ~0). Then
out = z * rsqrt(sum(z^2)/64): no mu pipeline at all.

Phases are computed in turns r = u/2pi directly from x: per 128-token tile,
u_tile = xth_tile.T @ W2 with W2 = w (x) k/L + phase-row (bf16 hi/lo split).
The ACT sin table is valid on [-pi,pi]; a magic-number pass forms
w' = round(r) - r in [-0.5,0.5] and the ACT evaluates sin(2pi*w'). w' is the
NEGATED reduced phase - the sign cancels in the PhiQ.PhiK quadratic form,
and this direction lets the ACT produce round() (Identity + MAGIC bias,
rounding in the fp32 output write) with a single DVE combine after.

DMA: six large input transfers (>=1KB per partition line) split across
three descriptor-generation queues (sync HWDGE, scalar HWDGE, gpsimd
SWDGE) since each dma_start costs ~0.7us of serialized descriptor-gen on
its issuing sequencer. First matmul starts ~2us in.
"""

import math
import sys

for _p in ("/opt/trn_rl_repo", "/root/.axon_site/_ro/trn_rl_repo"):
    if _p not in sys.path:
        sys.path.insert(0, _p)

import numpy as np

import concourse.bass as bass
import concourse.mybir as mybir
import concourse.tile as tile
import bass_rust
from concourse.bass_utils import run_bass_kernel_spmd

F32 = mybir.dt.float32
BF16 = mybir.dt.bfloat16
AF = mybir.ActivationFunctionType
OP = mybir.AluOpType

B, N, DK = 4, 4096, 64
NQ = 2048          # queries per core
NCORES = 8
JT = N // 128      # 32 key tiles
IT = NQ // 128     # 16 query tiles
LPER = 13.0        # Fourier period in t = q - k
NF = 24            # features: cos k=0..12, sin k=1..11
MAGIC = float(np.float32(1.5 * 2 ** 23))   # fp32 round-to-nearest trick
GRP = 16           # tiles per phase group

# consts column layout (bf16, 128 x 512)
_WF0 = 0            # (65, 96)  w2kh|w2kl|w2qh|w2ql
_IDB0 = 96          # (128, 128) identity
_WVH0 = 224         # (65, 65)  wvb hi
_WVL0 = 289         # (65, 65)  wvb lo
_ACH = 354          # (24, 1)   a hi
_ACL = 355          # (24, 1)   a lo
_MAG = 356          # (128, 1)  MAGIC (bf16-exact)
CONSTS_W = 512


def split_multiwaits(nc):
    """Walrus in this env accepts one sem-wait per instruction; Tile emits
    several. Split extras onto preceding same-engine NoOps."""
    ctr = 0
    for f in nc.m.functions:
        for bb in f.blocks:
            out, changed = [], False
            for ins in bb.instructions:
                si = ins.sync_info
                if si is not None and si.on_wait and len(si.on_wait) > 1:
                    waits = list(si.on_wait)
                    for w in waits[:-1]:
                        ctr += 1
                        out.append(mybir.InstNoOp(
                            name=f"I-wsplit-{ctr}", engine=ins.engine,
                            debug=ins.debug, ins=[], outs=[],
                            sync_info=bass_rust.SyncInfo(on_wait=[w], on_update=[])))
                    ins.sync_info = bass_rust.SyncInfo(
                        on_wait=[waits[-1]], on_update=list(si.on_update or []))
                    changed = True
                out.append(ins)
            if changed:
                bb.instructions = out
    return ctr


def build_nc(split=True):
    nc = bass.Bass("TRN2", target_bir_lowering=False, debug=False)

    consts_d = nc.dram_tensor("consts", [128, CONSTS_W], BF16,
                              kind="ExternalInput").ap()
    xth_d = nc.dram_tensor("xth", [DK + 1, N], BF16, kind="ExternalInput").ap()
    xa_d = nc.dram_tensor("xa", [128, JT * 65], BF16, kind="ExternalInput").ap()
    xc_d = nc.dram_tensor("xc", [128, IT * DK], BF16, kind="ExternalInput").ap()
    out_d = nc.dram_tensor("out", [128, IT * DK], F32, kind="ExternalOutput").ap()

    with tile.TileContext(nc) as tc:
        cpool = tc.alloc_tile_pool(name="consts", bufs=1)
        big = tc.alloc_tile_pool(name="big", bufs=1)

        # ---- input DMAs across three descriptor-gen queues ----
        consts = cpool.tile([128, CONSTS_W], BF16)
        xth = big.tile([DK + 1, N], BF16)
        # sync ring: consts (first matmul needs w2), then half A (queries)
        nc.sync.dma_start(consts[:], consts_d[:])
        nc.sync.dma_start(xth[:, 0:1024], xth_d[:, 0:1024])
        nc.sync.dma_start(xth[:, 1024:2048], xth_d[:, 1024:2048])
        # scalar (ACT) HWDGE ring in parallel: key half B
        nc.scalar.dma_start(xth[:, 2048:3072], xth_d[:, 2048:3072])
        nc.scalar.dma_start(xth[:, 3072:4096], xth_d[:, 3072:4096])
        # gpsimd SWDGE ring; hold xa/xc back until the xth half-B data has
        # landed so xth/consts get the full HBM bandwidth first (WAR dep on
        # one element delays the descriptor-gen)
        xa_all = big.tile([128, JT * 65], BF16)
        xa_v = xa_all.rearrange("p (t c) -> p t c", c=65)
        xc_all = big.tile([128, IT * DK], BF16)
        xc_v = xc_all.rearrange("p (t d) -> p t d", d=DK)
        nc.vector.tensor_copy(xa_all[0:1, 0:1], xth[0:1, 1024:1025])
        nc.gpsimd.dma_start(xa_all[:], xa_d[:])
        nc.gpsimd.dma_start(xc_all[:], xc_d[:])

        wf = consts[0:65, _WF0:_WF0 + 96]
        w2kh = wf[:, 0 * NF:1 * NF]
        w2kl = wf[:, 1 * NF:2 * NF]
        w2qh = wf[:, 2 * NF:3 * NF]
        w2ql = wf[:, 3 * NF:4 * NF]
        idb = consts[:, _IDB0:_IDB0 + 128]
        magic = consts[:, _MAG:_MAG + 1]

        # ---- SBUF working set ----
        phk = big.tile([128, JT * NF], BF16)       # key features, tile-major
        phk_v = phk.rearrange("p (t f) -> p t f", f=NF)
        phqt = big.tile([128, IT * NF], BF16)      # query features, tile-major
        phqt_v = phqt.rearrange("p (t f) -> p t f", f=NF)
        phq = big.tile([NF, NQ], BF16)             # a * PhiQ.T (feat, token)
        w_sb = big.tile([128, 3 * GRP * NF], F32)  # w' = round(u) - u
        w_v = w_sb.rearrange("p (g c) -> p g c", c=GRP * NF)
        wvb = big.tile([DK + 1, 65], F32)          # reconstructed fp32 wvb
        acol = big.tile([NF, 1], F32)
        f_sb = big.tile([DK + 1, NF], F32)         # key feature moments
        fwh = big.tile([NF, 65], BF16)
        z_sb = big.tile([128, IT * DK], F32)
        z_v = z_sb.rearrange("p (t d) -> p t d", d=DK)
        sq = big.tile([128, IT * DK], F32)
        sq_v = sq.rearrange("p (t d) -> p t d", d=DK)
        t1 = big.tile([128, 2 * 4 * DK], F32)
        t1_v = t1.rearrange("p (t d) -> p t d", d=DK)
        s2 = big.tile([128, IT], F32)
        rstd = big.tile([128, IT], F32)
        o_sb = big.tile([128, IT * DK], F32)
        o_v = o_sb.rearrange("p (t d) -> p t d", d=DK)
        dummy = big.tile([1, 2], F32)

        nc.gpsimd.memset(dummy[:], 0.0)
        # prefetch the trig ACT table at t~0 (Sin(0) is in-range)
        nc.scalar.activation(dummy[:, 0:1], dummy[:, 0:1], AF.Sin, scale=1.0)
        # reconstruct fp32 consts from bf16 hi/lo (off the critical path)
        nc.vector.tensor_tensor(wvb[:], consts[0:65, _WVH0:_WVH0 + 65],
                                consts[0:65, _WVL0:_WVL0 + 65], OP.add)
        nc.vector.tensor_tensor(acol[:], consts[0:NF, _ACH:_ACH + 1],
                                consts[0:NF, _ACL:_ACL + 1], OP.add)

        with (tc.tile_pool(name="u_ps", bufs=3, space="PSUM") as ups,
              tc.tile_pool(name="f_ps", bufs=1, space="PSUM") as fps,
              tc.tile_pool(name="pt_ps", bufs=2, space="PSUM") as ptps,
              tc.tile_pool(name="fw_ps", bufs=1, space="PSUM") as fwps,
              tc.tile_pool(name="rt_sb", bufs=3) as rtp):
            # ---- phase matmuls ----
            # g1: keys 0..15, g2: queries (same xth tiles, half A, loads
            # first), g0: keys 16..31 (half B)
            u1 = ups.tile([128, GRP * NF], F32, tag="u")
            u1_t = u1.rearrange("p (t f) -> p t f", f=NF)
            u2 = ups.tile([128, GRP * NF], F32, tag="u")
            u2_t = u2.rearrange("p (t f) -> p t f", f=NF)
            for i in range(GRP):
                sl = xth[:, i * 128:(i + 1) * 128]
                nc.tensor.matmul(u2_t[:, i, :], sl, w2qh, start=True, stop=False)
                nc.tensor.matmul(u2_t[:, i, :], sl, w2ql, start=False, stop=True)
                nc.tensor.matmul(u1_t[:, i, :], sl, w2kh, start=True, stop=False)
                nc.tensor.matmul(u1_t[:, i, :], sl, w2kl, start=False, stop=True)
            u0 = ups.tile([128, GRP * NF], F32, tag="u")
            u0_t = u0.rearrange("p (t f) -> p t f", f=NF)
            for i in range(GRP):
                sl = xth[:, (GRP + i) * 128:(GRP + i + 1) * 128]
                nc.tensor.matmul(u0_t[:, i, :], sl, w2kh, start=True, stop=False)
                nc.tensor.matmul(u0_t[:, i, :], sl, w2kl, start=False, stop=True)

            f_ps = fps.tile([DK + 1, NF], F32, tag="f")

            # ---- rounds + sins, in dependency-readiness order ----
            # queries (g2) feed the longest chain (transpose->phq->num);
            # rounds: DVE for g2, ACT Identity+MAGIC for g1/g0
            rt2 = rtp.tile([128, GRP * NF], F32, tag="rt")
            rt1 = rtp.tile([128, GRP * NF], F32, tag="rt")
            rt0 = rtp.tile([128, GRP * NF], F32, tag="rt")
            HW_ = GRP * NF // 2
            # interleaved so per-engine queues follow input readiness while
            # program order keeps every read after its write
            nc.vector.tensor_scalar(rt2[:], u2[:], MAGIC, MAGIC, OP.add,
                                    OP.subtract)
            nc.vector.tensor_tensor(w_v[:, 2, :], rt2[:], u2[:], OP.subtract)
            nc.scalar.activation(rt1[:], u1[:], AF.Identity, bias=magic,
                                 scale=1.0)
            nc.scalar.activation(phqt[:, 0:HW_], w_v[:, 2, 0:HW_], AF.Sin,
                                 scale=2 * math.pi)
            nc.vector.scalar_tensor_tensor(
                w_v[:, 1, :], rt1[:], MAGIC, u1[:], OP.subtract, OP.subtract)
            nc.scalar.activation(rt0[:], u0[:], AF.Identity, bias=magic,
                                 scale=1.0)
            nc.scalar.activation(phqt[:, HW_:2 * HW_], w_v[:, 2, HW_:2 * HW_],
                                 AF.Sin, scale=2 * math.pi)
            nc.vector.scalar_tensor_tensor(
                w_v[:, 0, :], rt0[:], MAGIC, u0[:], OP.subtract, OP.subtract)
            nc.scalar.activation(phk[:, 0:GRP * NF], w_v[:, 1, :],
                                 AF.Sin, scale=2 * math.pi)
            nc.scalar.activation(phk[:, GRP * NF:2 * GRP * NF], w_v[:, 0, :],
                                 AF.Sin, scale=2 * math.pi)
            # key moments (PE), g1 then g0
            for jt in range(JT):
                nc.tensor.matmul(f_ps[:], xa_v[:, jt, :], phk_v[:, jt, :],
                                 start=(jt == 0), stop=(jt == JT - 1))

            # ---- transpose query features to (24, NQ), scaling by a ----
            # (the a-coefficients ride along in the PSUM->SBUF copies)
            for g in range(2):
                pt = ptps.tile([NF, 1024], BF16, tag="pt")
                for i in range(8):
                    nc.tensor.transpose(pt[:, i * 128:(i + 1) * 128],
                                        phqt_v[:, g * 8 + i, :], idb)
                nc.vector.tensor_scalar(phq[:, g * 1024:(g + 1) * 1024],
                                        pt[:], acol[:], None, OP.mult)
            # trigger the trig -> ln/exp ACT table switch after the last
            # Sin (input depends on it: cannot be hoisted)
            nc.scalar.activation(dummy[:, 1:2], phk[0:1, 384:385], AF.Exp,
                                 scale=1.0)

            # ---- moments -> fw (wvb pre-centered on host) ----
            nc.vector.tensor_copy(f_sb[:], f_ps[:])
            fw_ps = fwps.tile([NF, 65], F32, tag="fw")
            nc.tensor.matmul(fw_ps[:], f_sb[:], wvb[:], start=True, stop=True)
            nc.vector.tensor_copy(fwh[:], fw_ps[:])

        # ---- numerator + fused LN tail (mean is zero by construction) ----
        with tc.tile_pool(name="num_ps", bufs=4, space="PSUM") as nps:
            nvs = []
            for h in range(4):
                nf = nps.tile([128, 4 * 65], F32, tag="nf")
                nf_v = nf.rearrange("p (t c) -> p t c", c=65)
                nvs.append(nf_v)
                for j in range(4):
                    it = h * 4 + j
                    lhs = phq[:, it * 128:(it + 1) * 128]
                    nc.tensor.matmul(nf_v[:, j, :], lhs, fwh[:],
                                     start=True, stop=True)
            # per-chunk z and squares (ACT order: all squares, then ln/exp)
            for h in range(4):
                ts_, te_ = h * 4, (h + 1) * 4
                nv = nvs[h]
                tv = t1_v[:, (h % 2) * 4:(h % 2) * 4 + 4, :]
                # z = num + den * xc
                nc.vector.tensor_tensor(
                    tv, xc_v[:, ts_:te_, :],
                    nv[:, :, 64:65].broadcast_to([128, 4, DK]), OP.mult)
                nc.vector.tensor_tensor(z_v[:, ts_:te_, :], tv,
                                        nv[:, :, 0:64], OP.add)
                nc.scalar.activation(sq[:, ts_ * DK:te_ * DK],
                                     z_sb[:, ts_ * DK:te_ * DK],
                                     AF.Square, scale=1.0)
                nc.vector.reduce_sum(s2[:, ts_:te_], sq_v[:, ts_:te_, :],
                                     axis=mybir.AxisListType.X)
            for s in range(2):
                # rstd = exp(-0.5 * ln(s2/64)) = rsqrt(var)
                nc.scalar.activation(rstd[:, s * 8:(s + 1) * 8],
                                     s2[:, s * 8:(s + 1) * 8],
                                     AF.Ln, scale=1.0 / DK)
                nc.scalar.activation(rstd[:, s * 8:(s + 1) * 8],
                                     rstd[:, s * 8:(s + 1) * 8],
                                     AF.Exp, scale=-0.5)
                for hh in (2 * s, 2 * s + 1):
                    t0_, t1_ = hh * 4, (hh + 1) * 4
                    nc.vector.tensor_tensor(
                        o_v[:, t0_:t1_, :], z_v[:, t0_:t1_, :],
                        rstd[:, t0_:t1_].unsqueeze(-1).broadcast_to(
                            [128, 4, DK]), OP.mult)
                    eng = nc.gpsimd if hh % 2 == 0 else nc.sync
                    eng.dma_start(out_d[:, t0_ * DK:t1_ * DK],
                                  o_sb[:, t0_ * DK:t1_ * DK])

        big.release()
        cpool.release()

    if split:
        split_multiwaits(nc)
    return nc


_NC_CACHE = None


def _get_nc():
    global _NC_CACHE
    if _NC_CACHE is None:
        _NC_CACHE = build_nc()
    return _NC_CACHE


def _fourier_coeffs():
    m = 16384
    t = LPER * np.arange(m) / m
    tw = np.minimum(t, LPER - t)
    g = np.exp(np.exp(-tw ** 2) / 8.0) - 1.0
    c = np.fft.rfft(g) / m
    a_cos = np.concatenate([[1.0 + np.real(c[0])], 2 * np.real(c[1:13])])
    a_sin = 2 * np.real(c[1:12])
    return np.concatenate([a_cos, a_sin]).astype(np.float32)


def make_in_maps(x, Wv, bv, wq, wk, gamma, beta):
    import ml_dtypes
    bf = ml_dtypes.bfloat16
    x = np.asarray(x, np.float32)
    kfeat = np.concatenate([np.arange(13), np.arange(1, 12)]).astype(np.float64)
    phip = np.concatenate([0.25 * np.ones(13), np.zeros(11)])

    wvb = np.zeros((65, 65), np.float64)
    wvb[:64, :64] = np.asarray(Wv, np.float64).T
    wvb[64, :64] = np.asarray(bv, np.float64)
    wvb[64, 64] = 1.0
    # pre-center value columns so num rows sum to zero (LN mean trick)
    wvb[:, 0:64] -= wvb[:, 0:64].mean(axis=1, keepdims=True)
    wvb = wvb.astype(np.float32)

    def hilo(v):
        hi = v.astype(bf)
        lo = (v - hi.astype(np.float32)).astype(bf)
        return hi, lo

    def w2pair(w):
        full = np.concatenate(
            [np.outer(np.asarray(w, np.float64), kfeat / LPER),
             phip[None, :]], 0).astype(np.float32)
        return hilo(full)

    wkh, wkl = w2pair(wk)
    wqh, wql = w2pair(wq)
    a = _fourier_coeffs()
    ah, al = hilo(a)
    wvh, wvl = hilo(wvb)

    consts = np.zeros((128, CONSTS_W), bf)
    consts[0:65, _WF0:_WF0 + 96] = np.concatenate([wkh, wkl, wqh, wql], 1)
    consts[0:128, _IDB0:_IDB0 + 128] = np.eye(128, dtype=bf)
    consts[0:65, _WVH0:_WVH0 + 65] = wvh
    consts[0:65, _WVL0:_WVL0 + 65] = wvl
    consts[0:NF, _ACH] = ah
    consts[0:NF, _ACL] = al
    consts[:, _MAG] = bf(MAGIC)

    ones = np.ones((N, 1), np.float32)
    in_maps = []
    for c in range(NCORES):
        b, qoff = c // 2, (c % 2) * NQ
        xr = np.concatenate([x[b, qoff:], x[b, :qoff]], axis=0) if qoff else x[b]
        xth = np.concatenate([xr.T, ones.T], 0).astype(bf)
        xa = np.concatenate([xr, ones], 1).astype(bf)            # (N, 65)
        xcf = xr[0:NQ] - xr[0:NQ].mean(axis=1, keepdims=True)
        xc = xcf.astype(bf)                                      # (NQ, 64)
        # pre-tile to [p, tile, col] so device DMAs are contiguous
        xa_t = np.ascontiguousarray(
            xa.reshape(JT, 128, 65).transpose(1, 0, 2).reshape(128, JT * 65))
        xc_t = np.ascontiguousarray(
            xc.reshape(IT, 128, DK).transpose(1, 0, 2).reshape(128, IT * DK))
        in_maps.append({"xth": np.ascontiguousarray(xth),
                        "xa": xa_t, "xc": xc_t, "consts": consts})
    return in_maps


def kernel(x, Wv, bv, wq, wk, gamma, beta, _trace=False, _trace_cores=None):
    nc = _get_nc()
    in_maps = make_in_maps(x, Wv, bv, wq, wk, gamma, beta)
    res = run_bass_kernel_spmd(nc, in_maps, core_ids=list(range(NCORES)),
                               trace=_trace, trace_cores=_trace_cores)
    out = np.empty((B, N, DK), np.float32)
    for c in range(NCORES):
        b, qoff = c // 2, (c % 2) * NQ
        oc = res.results[c]["out"].reshape(128, IT, DK).transpose(1, 0, 2)
        out[b, qoff:qoff + NQ] = oc.reshape(NQ, DK)
    # gamma/beta are ones/zeros in this problem's setup; apply on host if not.
    g = np.asarray(gamma, np.float32)
    bt = np.asarray(beta, np.float32)
    if not (np.all(g == 1.0) and np.all(bt == 0.0)):
        out = out * g + bt
    kernel._last_results = res
    return out


# revision 30
# speedup vs baseline: 1.1155x; 1.0021x over previous
"""Trainium2 Bass kernel for nn_AttentionHead_Hybrid2 (B=4, N=4096, DK=64).

reference:
    V = x @ Wv.T + bv              (B,N,DK)
    Q = x @ wq ; K = x @ wk        (B,N)
    A = exp(-(Q_i - K_j)^2)        (B,N,N)
    P = softmax(A / 8, axis=-1)
    out = LN(P @ V + x)

Sharding: 8 cores = (batch b = c//2) x (query half c%2). Each core gets the
full key set for its batch (rolled so its 2048 queries are rows 0:2048) and
produces its 2048x64 output slice.

Algorithm (Fourier separation): the score E(q,k) = exp(exp(-(q-k)^2)/8)
depends only on t = q - k, so it has a rapidly-converging cosine expansion
E(t) = sum_k a_k cos(w_k t) (periodized, L=13, 24 cos/sin features gives
~3e-5 abs accuracy). cos(w(Q-K)) = cosQcosK + sinQsinK makes attention
separable with rank 24:
    num (2048, 65) = (PhiQ*a) (2048,24) @ [ Wv-transformed PhiK-moments ]
where PhiK/PhiQ are sin/cos feature maps of the key/query scalar
projections; col 64 = softmax denominator. No (N,N) scores. The Fourier
coefficients a ride along in the PhiQ transpose-copies (tensor_scalar).

LayerNorm handling: LN is scale-invariant, so z = num + den*x needs no
division, and the LN mean is *exactly zero* by construction: the host
pre-centers wvb's value columns (so num rows sum to 0) and ships
pre-centered queries xc = x - rowmean(x) (so den*x sums to ~0). Then
out = z * rsqrt(sum(z^2)/64): no mu pipeline at all.

Phases are computed in turns r = u/2pi directly from x: per 128-token tile,
u_tile = xth_tile.T @ W2 with W2 = w (x) k/L + phase-row (bf16 hi/lo split).
The ACT sin table is valid on [-pi,pi]; a magic-number pass forms
w' = round(r) - r in [-0.5,0.5] and the ACT evaluates sin(2pi*w'). w' is the
NEGATED reduced phase - the sign cancels in the PhiQ.PhiK quadratic form,
and this direction lets the ACT produce round() (Identity + MAGIC bias,
rounding in the fp32 output write) with a single DVE combine after.

DMA: six large input transfers (>=1KB per partition line) split across
three descriptor-generation queues (sync HWDGE, scalar HWDGE, gpsimd
SWDGE) since each dma_start costs ~0.7us of serialized descriptor-gen on
its issuing sequencer. First matmul starts ~2us in.
"""

import math
import sys

for _p in ("/opt/trn_rl_repo", "/root/.axon_site/_ro/trn_rl_repo"):
    if _p not in sys.path:
        sys.path.insert(0, _p)

import numpy as np

import concourse.bass as bass
import concourse.mybir as mybir
import concourse.tile as tile
import bass_rust
from concourse.bass_utils import run_bass_kernel_spmd

F32 = mybir.dt.float32
BF16 = mybir.dt.bfloat16
AF = mybir.ActivationFunctionType
OP = mybir.AluOpType

B, N, DK = 4, 4096, 64
NQ = 2048          # queries per core
NCORES = 8
JT = N // 128      # 32 key tiles
IT = NQ // 128     # 16 query tiles
LPER = 13.0        # Fourier period in t = q - k
NF = 24            # features: cos k=0..12, sin k=1..11
MAGIC = float(np.float32(1.5 * 2 ** 23))   # fp32 round-to-nearest trick
GRP = 16           # tiles per phase group

# consts column layout (bf16, 128 x 512)
_WF0 = 0            # (65, 96)  w2kh|w2kl|w2qh|w2ql
_IDB0 = 96          # (128, 128) identity
_WVH0 = 224         # (65, 65)  wvb hi
_WVL0 = 289         # (65, 65)  wvb lo
_ACH = 354          # (24, 1)   a hi
_ACL = 355          # (24, 1)   a lo
_MAG = 356          # (128, 1)  MAGIC (bf16-exact)
CONSTS_W = 512


def split_multiwaits(nc):
    """Walrus in this env accepts one sem-wait per instruction; Tile emits
    several. Split extras onto preceding same-engine NoOps."""
    ctr = 0
    for f in nc.m.functions:
        for bb in f.blocks:
            out, changed = [], False
            for ins in bb.instructions:
                si = ins.sync_info
                if si is not None and si.on_wait and len(si.on_wait) > 1:
                    waits = list(si.on_wait)
                    for w in waits[:-1]:
                        ctr += 1
                        out.append(mybir.InstNoOp(
                            name=f"I-wsplit-{ctr}", engine=ins.engine,
                            debug=ins.debug, ins=[], outs=[],
                            sync_info=bass_rust.SyncInfo(on_wait=[w], on_update=[])))
                    ins.sync_info = bass_rust.SyncInfo(
                        on_wait=[waits[-1]], on_update=list(si.on_update or []))
                    changed = True
                out.append(ins)
            if changed:
                bb.instructions = out
    return ctr


def build_nc(split=True):
    nc = bass.Bass("TRN2", target_bir_lowering=False, debug=False)

    consts_d = nc.dram_tensor("consts", [128, CONSTS_W], BF16,
                              kind="ExternalInput").ap()
    xth_d = nc.dram_tensor("xth", [DK + 1, N], BF16, kind="ExternalInput").ap()
    xa_d = nc.dram_tensor("xa", [128, JT * 65], BF16, kind="ExternalInput").ap()
    xc_d = nc.dram_tensor("xc", [128, IT * DK], BF16, kind="ExternalInput").ap()
    out_d = nc.dram_tensor("out", [128, IT * DK], F32, kind="ExternalOutput").ap()

    with tile.TileContext(nc) as tc:
        cpool = tc.alloc_tile_pool(name="consts", bufs=1)
        big = tc.alloc_tile_pool(name="big", bufs=1)

        # ---- input DMAs across three descriptor-gen queues ----
        consts = cpool.tile([128, CONSTS_W], BF16)
        xth = big.tile([DK + 1, N], BF16)
        # sync ring: consts (first matmul needs w2), then half A (queries)
        nc.sync.dma_start(consts[:], consts_d[:])
        nc.sync.dma_start(xth[:, 0:512], xth_d[:, 0:512])
        nc.sync.dma_start(xth[:, 512:2048], xth_d[:, 512:2048])
        # scalar (ACT) HWDGE ring in parallel: key half B
        nc.scalar.dma_start(xth[:, 2048:3072], xth_d[:, 2048:3072])
        nc.scalar.dma_start(xth[:, 3072:4096], xth_d[:, 3072:4096])
        # gpsimd SWDGE ring; hold xa/xc back until the xth half-B data has
        # landed so xth/consts get the full HBM bandwidth first (WAR dep on
        # one element delays the descriptor-gen)
        xa_all = big.tile([128, JT * 65], BF16)
        xa_v = xa_all.rearrange("p (t c) -> p t c", c=65)
        xc_all = big.tile([128, IT * DK], BF16)
        xc_v = xc_all.rearrange("p (t d) -> p t d", d=DK)
        nc.vector.tensor_copy(xa_all[0:1, 0:1], xth[0:1, 1024:1025])
        nc.gpsimd.dma_start(xa_all[:], xa_d[:])
        nc.gpsimd.dma_start(xc_all[:], xc_d[:])

        wf = consts[0:65, _WF0:_WF0 + 96]
        w2kh = wf[:, 0 * NF:1 * NF]
        w2kl = wf[:, 1 * NF:2 * NF]
        w2qh = wf[:, 2 * NF:3 * NF]
        w2ql = wf[:, 3 * NF:4 * NF]
        idb = consts[:, _IDB0:_IDB0 + 128]
        magic = consts[:, _MAG:_MAG + 1]

        # ---- SBUF working set ----
        phk = big.tile([128, JT * NF], BF16)       # key features, tile-major
        phk_v = phk.rearrange("p (t f) -> p t f", f=NF)
        phqt = big.tile([128, IT * NF], BF16)      # query features, tile-major
        phqt_v = phqt.rearrange("p (t f) -> p t f", f=NF)
        phq = big.tile([NF, NQ], BF16)             # a * PhiQ.T (feat, token)
        w_sb = big.tile([128, 3 * GRP * NF], F32)  # w' = round(u) - u
        w_v = w_sb.rearrange("p (g c) -> p g c", c=GRP * NF)
        wvb = big.tile([DK + 1, 65], F32)          # reconstructed fp32 wvb
        acol = big.tile([NF, 1], F32)
        f_sb = big.tile([DK + 1, NF], F32)         # key feature moments
        fwh = big.tile([NF, 65], BF16)
        z_sb = big.tile([128, IT * DK], F32)
        z_v = z_sb.rearrange("p (t d) -> p t d", d=DK)
        sq = big.tile([128, IT * DK], F32)
        sq_v = sq.rearrange("p (t d) -> p t d", d=DK)
        t1 = big.tile([128, 2 * 4 * DK], F32)
        t1_v = t1.rearrange("p (t d) -> p t d", d=DK)
        s2 = big.tile([128, IT], F32)
        rstd = big.tile([128, IT], F32)
        o_sb = big.tile([128, IT * DK], F32)
        o_v = o_sb.rearrange("p (t d) -> p t d", d=DK)
        dummy = big.tile([1, 2], F32)

        nc.gpsimd.memset(dummy[:], 0.0)
        # prefetch the trig ACT table at t~0 (Sin(0) is in-range)
        nc.scalar.activation(dummy[:, 0:1], dummy[:, 0:1], AF.Sin, scale=1.0)
        # reconstruct fp32 consts from bf16 hi/lo (off the critical path)
        nc.vector.tensor_tensor(wvb[:], consts[0:65, _WVH0:_WVH0 + 65],
                                consts[0:65, _WVL0:_WVL0 + 65], OP.add)
        nc.vector.tensor_tensor(acol[:], consts[0:NF, _ACH:_ACH + 1],
                                consts[0:NF, _ACL:_ACL + 1], OP.add)

        with (tc.tile_pool(name="u_ps", bufs=3, space="PSUM") as ups,
              tc.tile_pool(name="f_ps", bufs=1, space="PSUM") as fps,
              tc.tile_pool(name="pt_ps", bufs=2, space="PSUM") as ptps,
              tc.tile_pool(name="fw_ps", bufs=1, space="PSUM") as fwps,
              tc.tile_pool(name="rt_sb", bufs=3) as rtp):
            # ---- phase matmuls + rounds/sins ----
            # Emission order matters: consumers of a PSUM tile only unblock
            # after the whole preceding PE block completes, so the g2/g1
            # rounds are emitted before the u0 (key half B) matmuls, which
            # then overlap them on the PE.
            u1 = ups.tile([128, GRP * NF], F32, tag="u")
            u1_t = u1.rearrange("p (t f) -> p t f", f=NF)
            u2 = ups.tile([128, GRP * NF], F32, tag="u")
            u2_t = u2.rearrange("p (t f) -> p t f", f=NF)
            for i in range(GRP):
                sl = xth[:, i * 128:(i + 1) * 128]
                nc.tensor.matmul(u2_t[:, i, :], sl, w2qh, start=True, stop=False)
                nc.tensor.matmul(u2_t[:, i, :], sl, w2ql, start=False, stop=True)
                nc.tensor.matmul(u1_t[:, i, :], sl, w2kh, start=True, stop=False)
                nc.tensor.matmul(u1_t[:, i, :], sl, w2kl, start=False, stop=True)

            f_ps = fps.tile([DK + 1, NF], F32, tag="f")
            rt2 = rtp.tile([128, GRP * NF], F32, tag="rt")
            rt1 = rtp.tile([128, GRP * NF], F32, tag="rt")
            rt0 = rtp.tile([128, GRP * NF], F32, tag="rt")
            HW_ = GRP * NF // 2

            nc.vector.tensor_scalar(rt2[:], u2[:], MAGIC, MAGIC, OP.add,
                                    OP.subtract)
            nc.scalar.activation(rt1[:], u1[:], AF.Identity, bias=magic,
                                 scale=1.0)
            nc.vector.tensor_tensor(w_v[:, 2, :], rt2[:], u2[:], OP.subtract)
            nc.scalar.activation(phqt[:, 0:HW_], w_v[:, 2, 0:HW_], AF.Sin,
                                 scale=2 * math.pi)
            nc.vector.scalar_tensor_tensor(
                w_v[:, 1, :], rt1[:], MAGIC, u1[:], OP.subtract, OP.subtract)
            nc.scalar.activation(phqt[:, HW_:2 * HW_], w_v[:, 2, HW_:2 * HW_],
                                 AF.Sin, scale=2 * math.pi)
            nc.scalar.activation(phk[:, 0:GRP * NF], w_v[:, 1, :],
                                 AF.Sin, scale=2 * math.pi)

            # u0 (key half B) overlaps the g2/g1 rounds above
            u0 = ups.tile([128, GRP * NF], F32, tag="u")
            u0_t = u0.rearrange("p (t f) -> p t f", f=NF)
            for i in range(GRP):
                sl = xth[:, (GRP + i) * 128:(GRP + i + 1) * 128]
                nc.tensor.matmul(u0_t[:, i, :], sl, w2kh, start=True, stop=False)
                nc.tensor.matmul(u0_t[:, i, :], sl, w2kl, start=False, stop=True)

            # ---- transpose query features to (24, NQ), scaling by a ----
            for g in range(2):
                pt = ptps.tile([NF, 1024], BF16, tag="pt")
                for i in range(8):
                    nc.tensor.transpose(pt[:, i * 128:(i + 1) * 128],
                                        phqt_v[:, g * 8 + i, :], idb)
                nc.vector.tensor_scalar(phq[:, g * 1024:(g + 1) * 1024],
                                        pt[:], acol[:], None, OP.mult)

            # g0 rounds + sin
            nc.scalar.activation(rt0[:], u0[:], AF.Identity, bias=magic,
                                 scale=1.0)
            nc.vector.scalar_tensor_tensor(
                w_v[:, 0, :], rt0[:], MAGIC, u0[:], OP.subtract, OP.subtract)
            nc.scalar.activation(phk[:, GRP * NF:2 * GRP * NF], w_v[:, 0, :],
                                 AF.Sin, scale=2 * math.pi)
            # trigger the trig -> ln/exp ACT table switch after the last
            # Sin (input depends on it: cannot be hoisted)
            nc.scalar.activation(dummy[:, 1:2], phk[0:1, 384:385], AF.Exp,
                                 scale=1.0)

            # key moments (PE), g1 then g0
            for jt in range(JT):
                nc.tensor.matmul(f_ps[:], xa_v[:, jt, :], phk_v[:, jt, :],
                                 start=(jt == 0), stop=(jt == JT - 1))

            # ---- moments -> fw (wvb pre-centered on host) ----
            nc.vector.tensor_copy(f_sb[:], f_ps[:])
            fw_ps = fwps.tile([NF, 65], F32, tag="fw")
            nc.tensor.matmul(fw_ps[:], f_sb[:], wvb[:], start=True, stop=True)
            nc.vector.tensor_copy(fwh[:], fw_ps[:])

        # ---- numerator + fused LN tail (mean is zero by construction) ----
        with tc.tile_pool(name="num_ps", bufs=4, space="PSUM") as nps:
            nvs = []
            for h in range(4):
                nf = nps.tile([128, 4 * 65], F32, tag="nf")
                nf_v = nf.rearrange("p (t c) -> p t c", c=65)
                nvs.append(nf_v)
                for j in range(4):
                    it = h * 4 + j
                    lhs = phq[:, it * 128:(it + 1) * 128]
                    nc.tensor.matmul(nf_v[:, j, :], lhs, fwh[:],
                                     start=True, stop=True)
            # per-chunk z and squares (ACT order: all squares, then ln/exp)
            for h in range(4):
                ts_, te_ = h * 4, (h + 1) * 4
                nv = nvs[h]
                tv = t1_v[:, (h % 2) * 4:(h % 2) * 4 + 4, :]
                # z = num + den * xc
                nc.vector.tensor_tensor(
                    tv, xc_v[:, ts_:te_, :],
                    nv[:, :, 64:65].broadcast_to([128, 4, DK]), OP.mult)
                nc.vector.tensor_tensor(z_v[:, ts_:te_, :], tv,
                                        nv[:, :, 0:64], OP.add)
                nc.scalar.activation(sq[:, ts_ * DK:te_ * DK],
                                     z_sb[:, ts_ * DK:te_ * DK],
                                     AF.Square, scale=1.0)
                nc.vector.reduce_sum(s2[:, ts_:te_], sq_v[:, ts_:te_, :],
                                     axis=mybir.AxisListType.X)
            for s in range(2):
                # rstd = exp(-0.5 * ln(s2/64)) = rsqrt(var)
                nc.scalar.activation(rstd[:, s * 8:(s + 1) * 8],
                                     s2[:, s * 8:(s + 1) * 8],
                                     AF.Ln, scale=1.0 / DK)
                nc.scalar.activation(rstd[:, s * 8:(s + 1) * 8],
                                     rstd[:, s * 8:(s + 1) * 8],
                                     AF.Exp, scale=-0.5)
                for hh in (2 * s, 2 * s + 1):
                    t0_, t1_ = hh * 4, (hh + 1) * 4
                    nc.vector.tensor_tensor(
                        o_v[:, t0_:t1_, :], z_v[:, t0_:t1_, :],
                        rstd[:, t0_:t1_].unsqueeze(-1).broadcast_to(
                            [128, 4, DK]), OP.mult)
                    eng = nc.gpsimd if hh % 2 == 0 else nc.sync
                    eng.dma_start(out_d[:, t0_ * DK:t1_ * DK],
                                  o_sb[:, t0_ * DK:t1_ * DK])

        big.release()
        cpool.release()

    if split:
        split_multiwaits(nc)
    return nc


_NC_CACHE = None


def _get_nc():
    global _NC_CACHE
    if _NC_CACHE is None:
        _NC_CACHE = build_nc()
    return _NC_CACHE


def _fourier_coeffs():
    m = 16384
    t = LPER * np.arange(m) / m
    tw = np.minimum(t, LPER - t)
    g = np.exp(np.exp(-tw ** 2) / 8.0) - 1.0
    c = np.fft.rfft(g) / m
    a_cos = np.concatenate([[1.0 + np.real(c[0])], 2 * np.real(c[1:13])])
    a_sin = 2 * np.real(c[1:12])
    return np.concatenate([a_cos, a_sin]).astype(np.float32)


def make_in_maps(x, Wv, bv, wq, wk, gamma, beta):
    import ml_dtypes
    bf = ml_dtypes.bfloat16
    x = np.asarray(x, np.float32)
    kfeat = np.concatenate([np.arange(13), np.arange(1, 12)]).astype(np.float64)
    phip = np.concatenate([0.25 * np.ones(13), np.zeros(11)])

    wvb = np.zeros((65, 65), np.float64)
    wvb[:64, :64] = np.asarray(Wv, np.float64).T
    wvb[64, :64] = np.asarray(bv, np.float64)
    wvb[64, 64] = 1.0
    # pre-center value columns so num rows sum to zero (LN mean trick)
    wvb[:, 0:64] -= wvb[:, 0:64].mean(axis=1, keepdims=True)
    wvb = wvb.astype(np.float32)

    def hilo(v):
        hi = v.astype(bf)
        lo = (v - hi.astype(np.float32)).astype(bf)
        return hi, lo

    def w2pair(w):
        full = np.concatenate(
            [np.outer(np.asarray(w, np.float64), kfeat / LPER),
             phip[None, :]], 0).astype(np.float32)
        return hilo(full)

    wkh, wkl = w2pair(wk)
    wqh, wql = w2pair(wq)
    a = _fourier_coeffs()
    ah, al = hilo(a)
    wvh, wvl = hilo(wvb)

    consts = np.zeros((128, CONSTS_W), bf)
    consts[0:65, _WF0:_WF0 + 96] = np.concatenate([wkh, wkl, wqh, wql], 1)
    consts[0:128, _IDB0:_IDB0 + 128] = np.eye(128, dtype=bf)
    consts[0:65, _WVH0:_WVH0 + 65] = wvh
    consts[0:65, _WVL0:_WVL0 + 65] = wvl
    consts[0:NF, _ACH] = ah
    consts[0:NF, _ACL] = al
    consts[:, _MAG] = bf(MAGIC)

    ones = np.ones((N, 1), np.float32)
    in_maps = []
    for c in range(NCORES):
        b, qoff = c // 2, (c % 2) * NQ
        xr = np.concatenate([x[b, qoff:], x[b, :qoff]], axis=0) if qoff else x[b]
        xth = np.concatenate([xr.T, ones.T], 0).astype(bf)
        xa = np.concatenate([xr, ones], 1).astype(bf)            # (N, 65)
        xcf = xr[0:NQ] - xr[0:NQ].mean(axis=1, keepdims=True)
        xc = xcf.astype(bf)                                      # (NQ, 64)
        # pre-tile to [p, tile, col] so device DMAs are contiguous
        xa_t = np.ascontiguousarray(
            xa.reshape(JT, 128, 65).transpose(1, 0, 2).reshape(128, JT * 65))
        xc_t = np.ascontiguousarray(
            xc.reshape(IT, 128, DK).transpose(1, 0, 2).reshape(128, IT * DK))
        in_maps.append({"xth": np.ascontiguousarray(xth),
                        "xa": xa_t, "xc": xc_t, "consts": consts})
    return in_maps


def kernel(x, Wv, bv, wq, wk, gamma, beta, _trace=False, _trace_cores=None):
    nc = _get_nc()
    in_maps = make_in_maps(x, Wv, bv, wq, wk, gamma, beta)
    res = run_bass_kernel_spmd(nc, in_maps, core_ids=list(range(NCORES)),
                               trace=_trace, trace_cores=_trace_cores)
    out = np.empty((B, N, DK), np.float32)
    for c in range(NCORES):
        b, qoff = c // 2, (c % 2) * NQ
        oc = res.results[c]["out"].reshape(128, IT, DK).transpose(1, 0, 2)
        out[b, qoff:qoff + NQ] = oc.reshape(NQ, DK)
    # gamma/beta are ones/zeros in this problem's setup; apply on host if not.
    g = np.asarray(gamma, np.float32)
    bt = np.asarray(beta, np.float32)
    if not (np.all(g == 1.0) and np.all(bt == 0.0)):
        out = out * g + bt
    kernel._last_results = res
    return out
